# revision 13
# baseline (speedup 1.0000x reference)
"""Multi-type GAT (node-level attention) kernel for Trainium2, 8 NeuronCores.

Strategy (graph partitioned by destination-node blocks of 128):
  * Host: per edge type, bucket edges by dst block (stable sort); within each
    bucket split edges by src half (< 32768) so every dma_gather call uses
    int16 indices into one half-table; assign buckets to cores balanced by
    tile count (LPT) within each (type, dst-half) group; build a uniform
    compile-time schedule so all 8 cores run one program.  The per-tile
    one-hot sel matrices (dst-local routing) are also host-built and shipped
    as bf16 inputs.
  * Device phase 1 (per type, interleaved with that type's phase 2):
    [h | es] = x @ [W | W a_src] per node tile on PE, rows stored bf16 to an
    internal DRAM table h_all[3*npadt, 256] (512B pitch).
  * Device phase 2, per superslot (4 dst-block slots, ~60-70 edge tiles):
      - dma_gather the 4 blocks' own rows; ONE batched mult+reduce gives
        ed_blk for all 4 slots
      - dma_gather [h|es][src] rows (512B each, by src half) for all tiles
      - per tile: PE-transpose the shipped sel, tiny matmul sts^T @ ed_blk
        accumulates ed per edge into one PSUM z strip [128, nt*4]
      - batched z+es add, Scalar-engine Prelu(0.2) and Exp
      - ONE batched rhs build (h*alpha | alpha) for all tiles
      - per tile: matmul psum[slot] += sel^T @ rhs, accumulated per slot
      - batched finalize: out = elu(agg / (denom + 1e-9)) for all 4 slots,
        single contiguous write per superslot
  * Host: unpermute slot-order rows back to node order.

The reference module computes the identical GAT stack twice (gat + gcn
branches), so the kernel computes once and returns the array twice.
"""

from contextlib import ExitStack

import numpy as np
import ml_dtypes

BF16 = ml_dtypes.bfloat16

P = 128
NEG_SLOPE = 0.2
HALF = 32768     # int16-addressable rows per gather table
SSG = 4          # buckets (slots) per superslot
STRIPE = 8       # node tiles per phase-1 stripe
ROWE = 256       # gather-row elements (bf16): [h 128 | es 4 | pad]


def _wrap_idx(vals):
    """dma_gather index packing: index i -> partition i%16, col i//16,
    replicated across the 8 groups of 16 partitions."""
    vals = np.asarray(vals, np.int16)
    assert len(vals) % 16 == 0
    w = vals.reshape(-1, 16).T
    return np.tile(w, (8, 1))


# ----------------------------------------------------------------------------
# host-side planning
# ----------------------------------------------------------------------------

def _plan(edges: np.ndarray, n_nodes: int, ncores: int):
    ntypes = edges.shape[0]
    nblk = (n_nodes + P - 1) // P
    npadt = ((nblk + STRIPE - 1) // STRIPE) * STRIPE * P
    nhblk = min(HALF // P, nblk)          # dst blocks in half 0

    # group buckets by (type, dst half); per bucket: src list split by src half
    groups = {}
    for t in range(ntypes):
        src = np.asarray(edges[t, 0], np.int64)
        dst = np.asarray(edges[t, 1], np.int64)
        blk = dst // P
        order = np.argsort(blk, kind="stable")
        bs, ss, ds_ = blk[order], src[order], dst[order]
        dl = ds_ - bs * P
        starts = np.searchsorted(bs, np.arange(nblk), "left")
        ends = np.searchsorted(bs, np.arange(nblk), "right")
        for bh in range(2):
            groups[(t, bh)] = []
        for b in range(nblk):
            sl = slice(starts[b], ends[b])
            sb, db = ss[sl], dl[sl]
            ah = sb < HALF
            bh = 0 if b < nhblk else 1
            groups[(t, bh)].append(
                (b, sb[ah], db[ah], sb[~ah] - HALF, db[~ah]))

    # LPT per group, then uniform schedule of (tA, tB) per rank
    plan_groups = []
    slot_id = 0
    outmap = [[] for _ in range(ncores)]
    for (t, bh), buckets in sorted(groups.items()):
        wt = [((len(x[1]) + P - 1) // P + (len(x[3]) + P - 1) // P)
              for x in buckets]
        order = np.argsort(-np.asarray(wt), kind="stable")
        cs = [[] for _ in range(ncores)]
        load = np.zeros(ncores, np.int64)
        for i in order:
            c = int(np.argmin(load))
            cs[c].append(int(i))
            load[c] += max(1, wt[i])
        S = max(len(x) for x in cs)
        S = ((S + SSG - 1) // SSG) * SSG
        ranks = []
        for r in range(S):
            ta = tb = 0
            for c in range(ncores):
                if r < len(cs[c]):
                    x = buckets[cs[c][r]]
                    ta = max(ta, (len(x[1]) + P - 1) // P)
                    tb = max(tb, (len(x[3]) + P - 1) // P)
            if ta + tb == 0:
                ta = 1
            ranks.append((ta, tb))
        for c in range(ncores):
            for r in range(S):
                if r < len(cs[c]):
                    outmap[c].append((t, buckets[cs[c][r]][0]))
                else:
                    outmap[c].append(None)
        plan_groups.append(dict(t=t, bh=bh, S=S, ranks=ranks, cs=cs,
                                buckets=buckets, slot0=slot_id))
        slot_id += S
    S_total = slot_id

    # compile-time tile stream + calls; per-core data arrays
    tiles = []      # (slot_id, first, last)
    calls = []      # dict(kind, t, src_half, num_idxs, woff, tile0)
    woff = 0        # int16 index-array column offset
    tile0 = 0
    core_idx = [[] for _ in range(ncores)]   # int16 stream per core
    core_blk = [[] for _ in range(ncores)]   # block-row idx stream
    core_dloc = [np.full((0,), 300.0, np.float32) for _ in range(ncores)]

    for g in plan_groups:
        t, bh, S, ranks, cs, buckets = (g["t"], g["bh"], g["S"], g["ranks"],
                                        g["cs"], g["buckets"])
        base_blk = 0 if bh == 0 else nhblk * P
        for s0 in range(0, S, SSG):
            rr = list(range(s0, min(s0 + SSG, S)))
            # block-row gather call for ed_blk (relative to dst-half base)
            calls.append(dict(kind="blk", t=t, src_half=bh,
                              num_idxs=len(rr) * P, woff=woff,
                              tile0=tile0, nt=len(rr),
                              slot0=g["slot0"] + s0))
            woff += len(rr) * P // 16
            for c in range(ncores):
                for r in rr:
                    if r < len(cs[c]):
                        b = buckets[cs[c][r]][0]
                        rel = b * P - base_blk
                    else:
                        rel = 0
                    core_blk[c].extend(range(rel, rel + P))
            for half, wcol in ((0, 1), (1, 3)):
                nt = sum(ranks[r][half] for r in rr)
                if nt == 0:
                    continue
                calls.append(dict(kind="edge", t=t, src_half=half,
                                  num_idxs=nt * P, woff=woff, tile0=tile0,
                                  nt=nt))
                woff += nt * P // 16
                for c in range(ncores):
                    seg_i = np.zeros(nt * P, np.int64)
                    seg_d = np.full(nt * P, 300.0, np.float32)
                    pos = 0
                    for r in rr:
                        trk = ranks[r][half]
                        if r < len(cs[c]):
                            x = buckets[cs[c][r]]
                            sv, dv = x[wcol], x[wcol + 1]
                            seg_i[pos:pos + len(sv)] = sv
                            seg_d[pos:pos + len(sv)] = dv
                        pos += trk * P
                    core_idx[c].append(seg_i)
                    core_dloc[c] = np.concatenate([core_dloc[c], seg_d])
                # tile bookkeeping
                for r in rr:
                    for j in range(ranks[r][half]):
                        sid = g["slot0"] + r
                        first = (half == 0 or ranks[r][0] == 0) and j == 0
                        last = ((half == 1 or ranks[r][1] == 0)
                                and j == ranks[r][half] - 1)
                        tiles.append((sid, first, last))
                        tile0 += 1

    tot_tiles = tile0
    W_total = woff

    # pack per-core arrays: gather indices + host-built one-hot sel
    sidx16 = np.zeros((ncores, 128, W_total), np.int16)
    selhost = np.zeros((ncores, 128, tot_tiles * P), BF16)
    for c in range(ncores):
        ei = 0
        blk_arr = np.asarray(core_blk[c], np.int64)
        bpos = 0
        for call in calls:
            n = call["num_idxs"]
            if call["kind"] == "blk":
                vals = blk_arr[bpos:bpos + n]
                bpos += n
            else:
                vals = core_idx[c][ei]
                ei += 1
            sidx16[c, :, call["woff"]:call["woff"] + n // 16] = _wrap_idx(vals)
        d = core_dloc[c].reshape(tot_tiles, P).astype(np.int64)
        oh = np.zeros((tot_tiles, P, P), BF16)
        ti, pp = np.nonzero((d >= 0) & (d < P))
        oh[ti, pp, d[ti, pp]] = 1.0
        selhost[c] = oh.transpose(1, 0, 2).reshape(P, tot_tiles * P)

    # max tiles in any superslot (PSUM z strip must fit one bank)
    nt_ss_max = 0
    i0 = 0
    while i0 < len(calls):
        assert calls[i0]["kind"] == "blk"
        j0 = i0 + 1
        acc = 0
        while j0 < len(calls) and calls[j0]["kind"] == "edge":
            acc += calls[j0]["nt"]
            j0 += 1
        nt_ss_max = max(nt_ss_max, acc)
        i0 = j0
    assert nt_ss_max * 4 <= 512, "z strip must fit one PSUM bank"

    return dict(ntypes=ntypes, nblk=nblk, npadt=npadt, nhblk=nhblk,
                S_total=S_total, tot_tiles=tot_tiles, W_total=W_total,
                tiles=tiles, calls=calls, outmap=outmap,
                sidx16=sidx16, selhost=selhost, nt_ss_max=nt_ss_max)


def _host_tensors(embedding, W, a_src, a_dst, plan):
    n, d = embedding.shape
    ntypes = W.shape[0]
    heads, hd = a_src.shape[1], a_src.shape[2]
    hk = heads * hd
    npadt = plan["npadt"]

    xT = np.zeros((d, npadt), np.float32)
    xT[:, :n] = np.asarray(embedding, np.float32).T
    xT = xT.astype(BF16)

    # Wmx[:, t*(hk+heads) : ...] = [W_t | W_t @ a_src_blockdiag]
    Wf = np.asarray(W, np.float32).reshape(ntypes, d, heads, hd)
    Was = np.einsum("tdhk,thk->tdh", Wf, np.asarray(a_src, np.float32))
    Wmx = np.concatenate(
        [Wf.reshape(ntypes, d, hk), Was], axis=2)      # [t, d, hk+heads]
    Wmx = np.ascontiguousarray(
        Wmx.transpose(1, 0, 2).reshape(d, ntypes * (hk + heads))).astype(BF16)

    adr = np.broadcast_to(
        np.asarray(a_dst, np.float32).reshape(ntypes, hk)
        .reshape(1, ntypes * hk), (P, ntypes * hk))
    adr = np.ascontiguousarray(adr).astype(BF16)

    ident = np.eye(P, dtype=np.float32).astype(BF16)
    return xT, Wmx, adr, ident


def _gather_compact(nc, mybir, out_ap, in_ap, idxs_ap, num_idxs, elem_size,
                    elem_step):
    """dma_gather with elem_size not a multiple of 256B (non-transpose,
    DRAM source). Mirrors BassGpSimd.dma_gather minus the transpose-only
    elem-size assert; row pitch (elem_step bytes) must stay 256B-aligned."""
    gp = nc.gpsimd
    assert idxs_ap.dtype == mybir.dt.int16
    dts = mybir.dt.size(in_ap.dtype)
    assert in_ap.ap[-1][1] == out_ap.ap[-1][1] == elem_size
    assert out_ap.ap[0][1] * out_ap.ap[1][1] == ((num_idxs + 127) // 128) * 128
    assert in_ap.ap[0][0] == elem_step
    stride_bytes = elem_step * dts
    assert stride_bytes % 256 == 0
    _in_ap = gp.lower_ap_dma(in_ap, for_custom_bir_dma=True)
    _idxs_ap = gp.lower_ap(idxs_ap)
    _out_ap = gp.lower_ap(out_ap)
    return gp.add_instruction(
        mybir.InstDMAGatherAnt(
            name=gp.bass.get_next_instruction_name(),
            ins=[*_in_ap, _idxs_ap,
                 gp.lower_val_access(gp.to_reg(num_idxs))],
            outs=[_out_ap],
            transpose=False,
            num_idxs=num_idxs,
            elem_size=elem_size,
            stride_bytes_256=stride_bytes // 256,
            gen_mode=0,
            single_packet=False,
            queue_num=0,
            sbuf_tokens_per_rank=0,
            sbuf_free_dim_per_rank=0,
            sbuf_free_dim_pad_per_rank=0,
            sbuf_byte_offset=0,
        )
    )


# ----------------------------------------------------------------------------
# device program
# ----------------------------------------------------------------------------

def _build_program(plan, d, heads, hd):
    import concourse.bacc as bacc
    import concourse.tile as tile
    import concourse.mybir as mybir

    dt = mybir.dt
    ntypes = plan["ntypes"]
    npadt = plan["npadt"]
    hk = heads * hd  # 128
    hx = hk + heads  # 132

    nc = bacc.Bacc("TRN2", target_bir_lowering=False, debug=False,
                   enable_asserts=False, num_devices=1)

    xT = nc.dram_tensor("xT", (d, npadt), dt.bfloat16, kind="ExternalInput")
    Wmx = nc.dram_tensor("Wmx", (d, ntypes * hx), dt.bfloat16,
                         kind="ExternalInput")
    adr = nc.dram_tensor("adr", (P, ntypes * hk), dt.bfloat16,
                         kind="ExternalInput")
    ident = nc.dram_tensor("ident", (P, P), dt.bfloat16, kind="ExternalInput")
    sidx = nc.dram_tensor("sidx", (128, plan["W_total"]), dt.int16,
                          kind="ExternalInput")
    selin = nc.dram_tensor("selin", (128, plan["tot_tiles"] * P), dt.bfloat16,
                           kind="ExternalInput")
    h_all = nc.dram_tensor("h_all", (ntypes * npadt, ROWE), dt.bfloat16,
                           kind="Internal")
    ycat = nc.dram_tensor("ycat", (plan["S_total"] * P, hk), dt.float32,
                          kind="ExternalOutput")

    nstripes = npadt // (STRIPE * P)
    AF = mybir.ActivationFunctionType
    OP = mybir.AluOpType
    tiles = plan["tiles"]
    nhblk = plan["nhblk"]
    calls = plan["calls"]

    with tile.TileContext(nc) as tc, ExitStack() as ctx:
        consts = ctx.enter_context(tc.tile_pool(name="consts", bufs=1))
        wsb = consts.tile([d, ntypes * hx], dt.bfloat16)
        nc.sync.dma_start(out=wsb[:], in_=Wmx.ap())
        adb = consts.tile([P, ntypes * hk], dt.bfloat16)
        nc.sync.dma_start(out=adb[:], in_=adr.ap())
        idsb = consts.tile([P, P], dt.bfloat16)
        nc.sync.dma_start(out=idsb[:], in_=ident.ap())
        zerob = consts.tile([P, 1], dt.float32)
        nc.vector.memset(zerob[:], 0.0)

        p1x = ctx.enter_context(tc.tile_pool(name="p1x", bufs=2))
        p1h = ctx.enter_context(tc.tile_pool(name="p1h", bufs=2))
        p1ps = ctx.enter_context(tc.tile_pool(name="p1ps", bufs=2,
                                              space="PSUM"))
        gidx = ctx.enter_context(tc.tile_pool(name="gidx", bufs=3))
        ghs = ctx.enter_context(tc.tile_pool(name="ghs", bufs=3))
        gblk = ctx.enter_context(tc.tile_pool(name="gblk", bufs=2))
        gsm = ctx.enter_context(tc.tile_pool(name="gsm", bufs=2))
        selp = ctx.enter_context(tc.tile_pool(name="selp", bufs=2))
        stsp = ctx.enter_context(tc.tile_pool(name="stsp", bufs=3))
        rhsp = ctx.enter_context(tc.tile_pool(name="rhsp", bufs=2))
        finp = ctx.enter_context(tc.tile_pool(name="finp", bufs=1))
        pst = ctx.enter_context(tc.tile_pool(name="pst", bufs=2, space="PSUM"))
        psz = ctx.enter_context(tc.tile_pool(name="psz", bufs=2, space="PSUM"))
        psa = ctx.enter_context(tc.tile_pool(name="psa", bufs=1, space="PSUM"))

        def phase1(t):
            for st in range(nstripes):
                base = st * STRIPE * P
                xt = p1x.tile([d, STRIPE * P], dt.bfloat16, tag="xt")
                nc.sync.dma_start(out=xt[:],
                                  in_=xT.ap()[:, base:base + STRIPE * P])
                hstr = p1h.tile([P, STRIPE * hx], dt.bfloat16, tag="hstr")
                for j in range(STRIPE):
                    lhs = xt[:, j * P:(j + 1) * P]
                    hp = p1ps.tile([P, hx], dt.float32)
                    nc.tensor.matmul(hp[:], lhs,
                                     wsb[:, t * hx:(t + 1) * hx],
                                     start=True, stop=True)
                    dst = hstr[:, j * hx:(j + 1) * hx]
                    if j % 2 == 0:
                        nc.scalar.copy(dst, hp[:])
                    else:
                        nc.vector.tensor_copy(dst, hp[:])
                out_ap = h_all.ap()[t * npadt + base:
                                    t * npadt + base + STRIPE * P, 0:hx]
                out_ap = out_ap.rearrange("(j p) k -> p j k", p=P)
                nc.sync.dma_start(out=out_ap, in_=hstr[:].rearrange(
                    "p (j k) -> p j k", k=hx))

        def superslot(ci):
            cblk = calls[ci]
            t = cblk["t"]
            base_row = t * npadt + cblk["src_half"] * nhblk * P
            lim = (nhblk * P if cblk["src_half"] == 0
                   else npadt - nhblk * P)
            nt_b = cblk["nt"]
            slot0 = cblk["slot0"]

            # ---- ed_blk for the slots: gather own-block rows, batched
            #      mult+reduce over the h columns
            it_b = gidx.tile([128, nt_b * P // 16], dt.int16, tag="itb")
            nc.sync.dma_start(
                out=it_b[:],
                in_=sidx.ap()[:, cblk["woff"]:cblk["woff"] + nt_b * P // 16])
            hb = gblk.tile([P, nt_b * hx], dt.bfloat16, tag="hb")
            _gather_compact(
                nc, mybir,
                out_ap=hb[:].rearrange("p (j k) -> p j k", k=hx),
                in_ap=h_all.ap()[base_row:base_row + lim, 0:hx],
                idxs_ap=it_b[:], num_idxs=nt_b * P, elem_size=hx,
                elem_step=ROWE)
            hb4 = hb[:].rearrange("p (s c) -> p s c", c=hx)
            tmpb = gsm.tile([P, nt_b * hk], dt.bfloat16, tag="tmpb")
            nc.vector.tensor_tensor(
                out=tmpb[:].rearrange("p (s h k) -> p s h k", h=heads, k=hd),
                in0=hb4[:, :, 0:hk].rearrange("p s (h k) -> p s h k", k=hd),
                in1=adb[:, t * hk:(t + 1) * hk]
                    .rearrange("p (h k) -> p h k", k=hd)
                    .unsqueeze(1).to_broadcast([P, nt_b, heads, hd]),
                op=OP.mult)
            edf = gsm.tile([P, nt_b * heads], dt.float32, tag="edf")
            nc.vector.tensor_reduce(
                out=edf[:].rearrange("p (s h) -> p s h", h=heads),
                in_=tmpb[:].rearrange("p (s h k) -> p s h k", h=heads, k=hd),
                axis=mybir.AxisListType.X, op=OP.add)
            edb = gsm.tile([P, nt_b * heads], dt.bfloat16, tag="edb")
            nc.vector.tensor_copy(edb[:], edf[:])

            # ---- edge calls of this superslot
            ss_edge = []
            cj = ci + 1
            while cj < len(calls) and calls[cj]["kind"] == "edge":
                ss_edge.append(calls[cj])
                cj += 1
            nt_ss = sum(cc["nt"] for cc in ss_edge)
            tile0_ss = ss_edge[0]["tile0"]

            hs = ghs.tile([P, nt_ss * hx], dt.bfloat16, tag="hs")
            off = 0
            for cc in ss_edge:
                base_e = t * npadt + cc["src_half"] * HALF
                lim_e = (min(HALF, npadt) if cc["src_half"] == 0
                         else npadt - HALF)
                nt = cc["nt"]
                it_e = gidx.tile([128, nt * P // 16], dt.int16, tag="ite",
                                 bufs=3)
                nc.sync.dma_start(
                    out=it_e[:],
                    in_=sidx.ap()[:, cc["woff"]:cc["woff"] + nt * P // 16])
                _gather_compact(
                    nc, mybir,
                    out_ap=hs[:, off * hx:(off + nt) * hx]
                        .rearrange("p (j k) -> p j k", k=hx),
                    in_ap=h_all.ap()[base_e:base_e + lim_e, 0:hx],
                    idxs_ap=it_e[:], num_idxs=nt * P, elem_size=hx,
                    elem_step=ROWE)
                off += nt
            hs4 = hs[:].rearrange("p (j c) -> p j c", c=hx)

            # ---- shipped one-hot sel for all tiles of the superslot
            sel = selp.tile([P, nt_ss * P], dt.bfloat16, tag="sel")
            nc.sync.dma_start(
                out=sel[:],
                in_=selin.ap()[:, tile0_ss * P:(tile0_ss + nt_ss) * P])

            # ---- per tile: transpose sel, ed matmul into z strip
            zps = psz.tile([P, nt_ss * heads], dt.float32, name="zps")
            for j in range(nt_ss):
                sid = tiles[tile0_ss + j][0]
                s_loc = sid - slot0
                stp = pst.tile([P, P], dt.bfloat16)
                nc.tensor.transpose(stp[:], sel[:, j * P:(j + 1) * P],
                                    idsb[:])
                sts = stsp.tile([P, P], dt.bfloat16, tag="sts")
                if j % 2 == 0:
                    nc.scalar.copy(sts[:], stp[:])
                else:
                    nc.vector.tensor_copy(sts[:], stp[:])
                nc.tensor.matmul(
                    zps[:, j * heads:(j + 1) * heads], sts[:],
                    edb[:, s_loc * heads:(s_loc + 1) * heads],
                    start=True, stop=True)

            # ---- batched alpha = exp(prelu(z + es, 0.2))
            zb = gsm.tile([P, nt_ss * heads], dt.float32, tag="zb")
            nc.vector.tensor_tensor(
                out=zb[:].rearrange("p (j h) -> p j h", h=heads),
                in0=zps[:].rearrange("p (j h) -> p j h", h=heads),
                in1=hs4[:, :, hk:hk + heads], op=OP.add)
            zl = gsm.tile([P, nt_ss * heads], dt.float32, tag="zl")
            nc.vector.scalar_tensor_tensor(
                out=zl[:], in0=zb[:], scalar=NEG_SLOPE, in1=zb[:],
                op0=OP.mult, op1=OP.max)
            ab = gsm.tile([P, nt_ss * heads], dt.bfloat16, tag="ab")
            nc.scalar.activation(ab[:], zl[:], AF.Exp)

            # ---- batched rhs build: [h*alpha | alpha]
            rhs = rhsp.tile([P, nt_ss * hx], dt.bfloat16, tag="rhs")
            rhs4 = rhs[:].rearrange("p (j c) -> p j c", c=hx)
            nc.vector.tensor_tensor(
                out=rhs4[:, :, 0:hk].rearrange("p j (h k) -> p j h k", k=hd),
                in0=hs4[:, :, 0:hk].rearrange("p j (h k) -> p j h k", k=hd),
                in1=ab[:].rearrange("p (j h) -> p j h", h=heads)
                    .unsqueeze(3).to_broadcast([P, nt_ss, heads, hd]),
                op=OP.mult)
            nc.vector.tensor_copy(
                rhs4[:, :, hk:hk + heads],
                ab[:].rearrange("p (j h) -> p j h", h=heads))

            # ---- per tile: aggregation matmul into per-slot psum region;
            #      grouped by slot so accumulation groups in a shared bank
            #      never overlap
            agg = psa.tile([P, SSG * 256], dt.float32, name="aggps")
            by_slot = [[] for _ in range(SSG)]
            for j in range(nt_ss):
                sid = tiles[tile0_ss + j][0]
                by_slot[sid - slot0].append(j)
            for s_loc in range(SSG):
                for j in by_slot[s_loc]:
                    _, first, last = tiles[tile0_ss + j]
                    nc.tensor.matmul(
                        agg[:, s_loc * 256:s_loc * 256 + hx],
                        sel[:, j * P:(j + 1) * P],
                        rhs4[:, j, :],
                        start=first, stop=last)

            # ---- batched finalize: out = elu(agg / (denom + 1e-9))
            agg4 = agg[:].rearrange("p (s c) -> p s c", c=256)
            dn = finp.tile([P, SSG * heads], dt.float32, tag="dn")
            nc.vector.tensor_scalar_add(
                dn[:].rearrange("p (s h) -> p s h", h=heads),
                agg4[:, :, hk:hk + heads], 1e-9)
            rc = finp.tile([P, SSG * heads], dt.float32, tag="rc")
            nc.vector.reciprocal(rc[:], dn[:])
            of = finp.tile([P, SSG * hk], dt.float32, tag="of")
            nc.vector.tensor_tensor(
                out=of[:].rearrange("p (s h k) -> p s h k", h=heads, k=hd),
                in0=agg4[:, :, 0:hk].rearrange("p s (h k) -> p s h k", k=hd),
                in1=rc[:].rearrange("p (s h) -> p s h", h=heads)
                    .unsqueeze(3).to_broadcast([P, SSG, heads, hd]),
                op=OP.mult)
            # elu(x) = (min(exp(x), 1) + max(x, 0)) - 1
            ex = finp.tile([P, SSG * hk], dt.float32, tag="ex")
            nc.scalar.activation(ex[:], of[:], AF.Exp)
            exm = finp.tile([P, SSG * hk], dt.float32, tag="exm")
            nc.vector.scalar_tensor_tensor(
                out=exm[:], in0=ex[:], scalar=1.0,
                in1=zerob[:].to_broadcast([P, SSG * hk]),
                op0=OP.min, op1=OP.add)
            s2 = finp.tile([P, SSG * hk], dt.float32, tag="s2")
            nc.vector.scalar_tensor_tensor(
                out=s2[:], in0=of[:], scalar=0.0, in1=exm[:],
                op0=OP.max, op1=OP.add)
            ysb = finp.tile([P, SSG * hk], dt.float32, tag="ysb")
            nc.vector.scalar_tensor_tensor(
                out=ysb[:], in0=s2[:], scalar=-1.0,
                in1=zerob[:].to_broadcast([P, SSG * hk]),
                op0=OP.add, op1=OP.add)
            nc.sync.dma_start(
                out=ycat.ap()[slot0 * P:(slot0 + SSG) * P, :]
                    .rearrange("(s p) k -> p s k", p=P),
                in_=ysb[:].rearrange("p (s k) -> p s k", k=hk))
            return cj

        # interleave: phase 1 of type t, then its superslots (types are
        # contiguous in the call stream)
        ci = 0
        for t in range(ntypes):
            phase1(t)
            while ci < len(calls) and calls[ci]["t"] == t:
                ci = superslot(ci)
        assert ci == len(calls)

    nc.compile()
    return nc


# ----------------------------------------------------------------------------
# public entry
# ----------------------------------------------------------------------------

def _run(embedding, edges, W, a_src, a_dst, ncores=8, sim=False, trace=False):
    embedding = np.asarray(embedding, np.float32)
    edges = np.asarray(edges, np.int32)
    W = np.asarray(W, np.float32)
    a_src = np.asarray(a_src, np.float32)
    a_dst = np.asarray(a_dst, np.float32)

    n, d = embedding.shape
    ntypes = edges.shape[0]
    heads, hd = a_src.shape[1], a_src.shape[2]

    plan = _plan(edges, n, ncores)
    xT, Wmx, adr, ident = _host_tensors(embedding, W, a_src, a_dst, plan)
    nc = _build_program(plan, d, heads, hd)

    in_maps = []
    for c in range(ncores):
        in_maps.append({
            "xT": xT, "Wmx": Wmx, "adr": adr, "ident": ident,
            "sidx": plan["sidx16"][c], "selin": plan["selhost"][c],
        })

    if sim:
        from concourse.bass_interp import CoreSim
        results = []
        for c in range(ncores):
            s = CoreSim(nc)
            for k, v in in_maps[c].items():
                s.tensor(k)[:] = v
            s.simulate()
            results.append({"ycat": np.array(s.tensor("ycat"))})
        exec_ns = None
    else:
        from concourse.bass_utils import run_bass_kernel_spmd
        r = run_bass_kernel_spmd(nc, in_maps, core_ids=list(range(ncores)),
                                 trace=trace)
        results = r.results
        exec_ns = r.exec_time_ns
        if trace:
            _TRACE[0] = r

    out = np.zeros((ntypes, n, heads * hd), np.float32)
    for c in range(ncores):
        y = results[c]["ycat"]
        for sid, tb in enumerate(plan["outmap"][c]):
            if tb is None:
                continue
            t, b = tb
            lo = b * P
            hi = min(n, lo + P)
            out[t, lo:hi, :] = y[sid * P:sid * P + (hi - lo), :]
    return out, exec_ns


_EXEC_NS = [None]
_TRACE = [None]


def kernel(embedding, edges, W, a_src, a_dst):
    out, exec_ns = _run(embedding, edges, W, a_src, a_dst, ncores=8, sim=False)
    _EXEC_NS[0] = exec_ns
    return out, out.copy()


# revision 14
# speedup vs baseline: 1.2680x; 1.2680x over previous
"""Multi-type GAT (node-level attention) kernel for Trainium2, 8 NeuronCores.

Strategy (graph partitioned by destination-node blocks of 128):
  * Host: per edge type, bucket edges by dst block (stable sort); within each
    bucket split edges by src half (< 32768) so every dma_gather call uses
    int16 indices into one half-table; assign buckets to cores balanced by
    tile count (LPT) within each (type, dst-half) group; build a uniform
    compile-time schedule so all 8 cores run one program.  The per-tile
    one-hot sel matrices (dst-local routing) are also host-built and shipped
    as bf16 inputs.
  * Device phase 1 (per type, interleaved with that type's phase 2):
    [h | es] = x @ [W | W a_src] per node tile on PE, rows stored bf16 to an
    internal DRAM table h_all[3*npadt, 256] (512B pitch).
  * Device phase 2, per superslot (4 dst-block slots, ~60-70 edge tiles):
      - dma_gather the 4 blocks' own rows; ONE batched mult+reduce gives
        ed_blk for all 4 slots
      - dma_gather [h|es][src] rows (512B each, by src half) for all tiles
      - per tile: PE-transpose the shipped sel, tiny matmul sts^T @ ed_blk
        accumulates ed per edge into one PSUM z strip [128, nt*4]
      - batched z+es add, Scalar-engine Prelu(0.2) and Exp
      - ONE batched rhs build (h*alpha | alpha) for all tiles
      - per tile: matmul psum[slot] += sel^T @ rhs, accumulated per slot
      - batched finalize: out = elu(agg / (denom + 1e-9)) for all 4 slots,
        single contiguous write per superslot
  * Host: unpermute slot-order rows back to node order.

The reference module computes the identical GAT stack twice (gat + gcn
branches), so the kernel computes once and returns the array twice.
"""

from contextlib import ExitStack

import numpy as np
import ml_dtypes

BF16 = ml_dtypes.bfloat16

P = 128
NEG_SLOPE = 0.2
HALF = 32768     # int16-addressable rows per gather table
SSG = 4          # buckets (slots) per superslot
STRIPE = 8       # node tiles per phase-1 stripe
ROWE = 256       # gather-row elements (bf16): [h 128 | es 4 | pad]


def _wrap_idx(vals):
    """dma_gather index packing: index i -> partition i%16, col i//16,
    replicated across the 8 groups of 16 partitions."""
    vals = np.asarray(vals, np.int16)
    assert len(vals) % 16 == 0
    w = vals.reshape(-1, 16).T
    return np.tile(w, (8, 1))


# ----------------------------------------------------------------------------
# host-side planning
# ----------------------------------------------------------------------------

def _plan(edges: np.ndarray, n_nodes: int, ncores: int):
    ntypes = edges.shape[0]
    nblk = (n_nodes + P - 1) // P
    npadt = ((nblk + STRIPE - 1) // STRIPE) * STRIPE * P
    nhblk = min(HALF // P, nblk)          # dst blocks in half 0

    # group buckets by (type, dst half); per bucket: src list split by src half
    groups = {}
    for t in range(ntypes):
        src = np.asarray(edges[t, 0], np.int64)
        dst = np.asarray(edges[t, 1], np.int64)
        blk = dst // P
        order = np.argsort(blk, kind="stable")
        bs, ss, ds_ = blk[order], src[order], dst[order]
        dl = ds_ - bs * P
        starts = np.searchsorted(bs, np.arange(nblk), "left")
        ends = np.searchsorted(bs, np.arange(nblk), "right")
        for bh in range(2):
            groups[(t, bh)] = []
        for b in range(nblk):
            sl = slice(starts[b], ends[b])
            sb, db = ss[sl], dl[sl]
            ah = sb < HALF
            bh = 0 if b < nhblk else 1
            groups[(t, bh)].append(
                (b, sb[ah], db[ah], sb[~ah] - HALF, db[~ah]))

    # LPT per group, then uniform schedule of (tA, tB) per rank
    plan_groups = []
    slot_id = 0
    outmap = [[] for _ in range(ncores)]
    for (t, bh), buckets in sorted(groups.items()):
        wt = [((len(x[1]) + P - 1) // P + (len(x[3]) + P - 1) // P)
              for x in buckets]
        order = np.argsort(-np.asarray(wt), kind="stable")
        cs = [[] for _ in range(ncores)]
        load = np.zeros(ncores, np.int64)
        for i in order:
            c = int(np.argmin(load))
            cs[c].append(int(i))
            load[c] += max(1, wt[i])
        S = max(len(x) for x in cs)
        S = ((S + SSG - 1) // SSG) * SSG
        ranks = []
        for r in range(S):
            ta = tb = 0
            for c in range(ncores):
                if r < len(cs[c]):
                    x = buckets[cs[c][r]]
                    ta = max(ta, (len(x[1]) + P - 1) // P)
                    tb = max(tb, (len(x[3]) + P - 1) // P)
            if ta + tb == 0:
                ta = 1
            ranks.append((ta, tb))
        for c in range(ncores):
            for r in range(S):
                if r < len(cs[c]):
                    outmap[c].append((t, buckets[cs[c][r]][0]))
                else:
                    outmap[c].append(None)
        plan_groups.append(dict(t=t, bh=bh, S=S, ranks=ranks, cs=cs,
                                buckets=buckets, slot0=slot_id))
        slot_id += S
    S_total = slot_id

    # compile-time tile stream + calls; per-core data arrays
    tiles = []      # (slot_id, first, last)
    calls = []      # dict(kind, t, src_half, num_idxs, woff, tile0)
    woff = 0        # int16 index-array column offset
    tile0 = 0
    core_idx = [[] for _ in range(ncores)]   # int16 stream per core
    core_blk = [[] for _ in range(ncores)]   # block-row idx stream
    core_dloc = [np.full((0,), 300.0, np.float32) for _ in range(ncores)]

    for g in plan_groups:
        t, bh, S, ranks, cs, buckets = (g["t"], g["bh"], g["S"], g["ranks"],
                                        g["cs"], g["buckets"])
        base_blk = 0 if bh == 0 else nhblk * P
        for s0 in range(0, S, SSG):
            rr = list(range(s0, min(s0 + SSG, S)))
            # block-row gather call for ed_blk (relative to dst-half base)
            calls.append(dict(kind="blk", t=t, src_half=bh,
                              num_idxs=len(rr) * P, woff=woff,
                              tile0=tile0, nt=len(rr),
                              slot0=g["slot0"] + s0))
            woff += len(rr) * P // 16
            for c in range(ncores):
                for r in rr:
                    if r < len(cs[c]):
                        b = buckets[cs[c][r]][0]
                        rel = b * P - base_blk
                    else:
                        rel = 0
                    core_blk[c].extend(range(rel, rel + P))
            for half, wcol in ((0, 1), (1, 3)):
                nt = sum(ranks[r][half] for r in rr)
                if nt == 0:
                    continue
                calls.append(dict(kind="edge", t=t, src_half=half,
                                  num_idxs=nt * P, woff=woff, tile0=tile0,
                                  nt=nt))
                woff += nt * P // 16
                for c in range(ncores):
                    seg_i = np.zeros(nt * P, np.int64)
                    seg_d = np.full(nt * P, 300.0, np.float32)
                    pos = 0
                    for r in rr:
                        trk = ranks[r][half]
                        if r < len(cs[c]):
                            x = buckets[cs[c][r]]
                            sv, dv = x[wcol], x[wcol + 1]
                            seg_i[pos:pos + len(sv)] = sv
                            seg_d[pos:pos + len(sv)] = dv
                        pos += trk * P
                    core_idx[c].append(seg_i)
                    core_dloc[c] = np.concatenate([core_dloc[c], seg_d])
                # tile bookkeeping
                for r in rr:
                    for j in range(ranks[r][half]):
                        sid = g["slot0"] + r
                        first = (half == 0 or ranks[r][0] == 0) and j == 0
                        last = ((half == 1 or ranks[r][1] == 0)
                                and j == ranks[r][half] - 1)
                        tiles.append((sid, first, last))
                        tile0 += 1

    tot_tiles = tile0
    W_total = woff

    # pack per-core arrays: gather indices + host-built one-hot sel
    sidx16 = np.zeros((ncores, 128, W_total), np.int16)
    selhost = np.zeros((ncores, 128, tot_tiles * P), BF16)
    for c in range(ncores):
        ei = 0
        blk_arr = np.asarray(core_blk[c], np.int64)
        bpos = 0
        for call in calls:
            n = call["num_idxs"]
            if call["kind"] == "blk":
                vals = blk_arr[bpos:bpos + n]
                bpos += n
            else:
                vals = core_idx[c][ei]
                ei += 1
            sidx16[c, :, call["woff"]:call["woff"] + n // 16] = _wrap_idx(vals)
        d = core_dloc[c].reshape(tot_tiles, P).astype(np.int64)
        oh = np.zeros((tot_tiles, P, P), BF16)
        ti, pp = np.nonzero((d >= 0) & (d < P))
        oh[ti, pp, d[ti, pp]] = 1.0
        selhost[c] = oh.transpose(1, 0, 2).reshape(P, tot_tiles * P)

    # max tiles in any superslot (PSUM z strip must fit one bank)
    nt_ss_max = 0
    i0 = 0
    while i0 < len(calls):
        assert calls[i0]["kind"] == "blk"
        j0 = i0 + 1
        acc = 0
        while j0 < len(calls) and calls[j0]["kind"] == "edge":
            acc += calls[j0]["nt"]
            j0 += 1
        nt_ss_max = max(nt_ss_max, acc)
        i0 = j0
    assert nt_ss_max * 4 <= 512, "z strip must fit one PSUM bank"

    return dict(ntypes=ntypes, nblk=nblk, npadt=npadt, nhblk=nhblk,
                S_total=S_total, tot_tiles=tot_tiles, W_total=W_total,
                tiles=tiles, calls=calls, outmap=outmap,
                sidx16=sidx16, selhost=selhost, nt_ss_max=nt_ss_max)


def _host_tensors(embedding, W, a_src, a_dst, plan):
    n, d = embedding.shape
    ntypes = W.shape[0]
    heads, hd = a_src.shape[1], a_src.shape[2]
    hk = heads * hd
    npadt = plan["npadt"]

    xT = np.zeros((d, npadt), np.float32)
    xT[:, :n] = np.asarray(embedding, np.float32).T
    xT = xT.astype(BF16)

    # Wmx[:, t*(hk+heads) : ...] = [W_t | W_t @ a_src_blockdiag]
    Wf = np.asarray(W, np.float32).reshape(ntypes, d, heads, hd)
    Was = np.einsum("tdhk,thk->tdh", Wf, np.asarray(a_src, np.float32))
    Wmx = np.concatenate(
        [Wf.reshape(ntypes, d, hk), Was], axis=2)      # [t, d, hk+heads]
    Wmx = np.ascontiguousarray(
        Wmx.transpose(1, 0, 2).reshape(d, ntypes * (hk + heads))).astype(BF16)

    adr = np.broadcast_to(
        np.asarray(a_dst, np.float32).reshape(ntypes, hk)
        .reshape(1, ntypes * hk), (P, ntypes * hk))
    adr = np.ascontiguousarray(adr).astype(BF16)

    ident = np.eye(P, dtype=np.float32).astype(BF16)
    return xT, Wmx, adr, ident


def _gather_compact(nc, mybir, out_ap, in_ap, idxs_ap, num_idxs, elem_size,
                    elem_step):
    """dma_gather with elem_size not a multiple of 256B (non-transpose,
    DRAM source). Mirrors BassGpSimd.dma_gather minus the transpose-only
    elem-size assert; row pitch (elem_step bytes) must stay 256B-aligned."""
    gp = nc.gpsimd
    assert idxs_ap.dtype == mybir.dt.int16
    dts = mybir.dt.size(in_ap.dtype)
    assert in_ap.ap[-1][1] == out_ap.ap[-1][1] == elem_size
    assert out_ap.ap[0][1] * out_ap.ap[1][1] == ((num_idxs + 127) // 128) * 128
    assert in_ap.ap[0][0] == elem_step
    stride_bytes = elem_step * dts
    assert stride_bytes % 256 == 0
    _in_ap = gp.lower_ap_dma(in_ap, for_custom_bir_dma=True)
    _idxs_ap = gp.lower_ap(idxs_ap)
    _out_ap = gp.lower_ap(out_ap)
    return gp.add_instruction(
        mybir.InstDMAGatherAnt(
            name=gp.bass.get_next_instruction_name(),
            ins=[*_in_ap, _idxs_ap,
                 gp.lower_val_access(gp.to_reg(num_idxs))],
            outs=[_out_ap],
            transpose=False,
            num_idxs=num_idxs,
            elem_size=elem_size,
            stride_bytes_256=stride_bytes // 256,
            gen_mode=0,
            single_packet=False,
            queue_num=0,
            sbuf_tokens_per_rank=0,
            sbuf_free_dim_per_rank=0,
            sbuf_free_dim_pad_per_rank=0,
            sbuf_byte_offset=0,
        )
    )


# ----------------------------------------------------------------------------
# device program
# ----------------------------------------------------------------------------

def _build_program(plan, d, heads, hd):
    import concourse.bacc as bacc
    import concourse.tile as tile
    import concourse.mybir as mybir

    dt = mybir.dt
    ntypes = plan["ntypes"]
    npadt = plan["npadt"]
    hk = heads * hd  # 128
    hx = hk + heads  # 132

    nc = bacc.Bacc("TRN2", target_bir_lowering=False, debug=False,
                   enable_asserts=False, num_devices=1)

    xT = nc.dram_tensor("xT", (d, npadt), dt.bfloat16, kind="ExternalInput")
    Wmx = nc.dram_tensor("Wmx", (d, ntypes * hx), dt.bfloat16,
                         kind="ExternalInput")
    adr = nc.dram_tensor("adr", (P, ntypes * hk), dt.bfloat16,
                         kind="ExternalInput")
    ident = nc.dram_tensor("ident", (P, P), dt.bfloat16, kind="ExternalInput")
    sidx = nc.dram_tensor("sidx", (128, plan["W_total"]), dt.int16,
                          kind="ExternalInput")
    selin = nc.dram_tensor("selin", (128, plan["tot_tiles"] * P), dt.bfloat16,
                           kind="ExternalInput")
    h_all = nc.dram_tensor("h_all", (ntypes * npadt, ROWE), dt.bfloat16,
                           kind="Internal")
    ycat = nc.dram_tensor("ycat", (plan["S_total"] * P, hk), dt.float32,
                          kind="ExternalOutput")

    nstripes = npadt // (STRIPE * P)
    AF = mybir.ActivationFunctionType
    OP = mybir.AluOpType
    tiles = plan["tiles"]
    nhblk = plan["nhblk"]
    calls = plan["calls"]

    with tile.TileContext(nc) as tc, ExitStack() as ctx:
        consts = ctx.enter_context(tc.tile_pool(name="consts", bufs=1))
        wsb = consts.tile([d, ntypes * hx], dt.bfloat16)
        nc.sync.dma_start(out=wsb[:], in_=Wmx.ap())
        adb = consts.tile([P, ntypes * hk], dt.bfloat16)
        nc.sync.dma_start(out=adb[:], in_=adr.ap())
        idsb = consts.tile([P, P], dt.bfloat16)
        nc.sync.dma_start(out=idsb[:], in_=ident.ap())
        zerob = consts.tile([P, 1], dt.float32)
        nc.vector.memset(zerob[:], 0.0)

        p1x = ctx.enter_context(tc.tile_pool(name="p1x", bufs=2))
        p1h = ctx.enter_context(tc.tile_pool(name="p1h", bufs=2))
        p1ps = ctx.enter_context(tc.tile_pool(name="p1ps", bufs=2,
                                              space="PSUM"))
        gidx = ctx.enter_context(tc.tile_pool(name="gidx", bufs=3))
        ghs = ctx.enter_context(tc.tile_pool(name="ghs", bufs=2))
        gblk = ctx.enter_context(tc.tile_pool(name="gblk", bufs=2))
        gsm = ctx.enter_context(tc.tile_pool(name="gsm", bufs=2))
        selp = ctx.enter_context(tc.tile_pool(name="selp", bufs=2))
        stsp = ctx.enter_context(tc.tile_pool(name="stsp", bufs=3))
        rhsp = ctx.enter_context(tc.tile_pool(name="rhsp", bufs=2))
        finp = ctx.enter_context(tc.tile_pool(name="finp", bufs=1))
        pst = ctx.enter_context(tc.tile_pool(name="pst", bufs=2, space="PSUM"))
        psz = ctx.enter_context(tc.tile_pool(name="psz", bufs=2, space="PSUM"))
        psa = ctx.enter_context(tc.tile_pool(name="psa", bufs=1, space="PSUM"))

        def phase1(t):
            for st in range(nstripes):
                base = st * STRIPE * P
                xt = p1x.tile([d, STRIPE * P], dt.bfloat16, tag="xt")
                nc.sync.dma_start(out=xt[:],
                                  in_=xT.ap()[:, base:base + STRIPE * P])
                hstr = p1h.tile([P, STRIPE * hx], dt.bfloat16, tag="hstr")
                for j in range(STRIPE):
                    lhs = xt[:, j * P:(j + 1) * P]
                    hp = p1ps.tile([P, hx], dt.float32)
                    nc.tensor.matmul(hp[:], lhs,
                                     wsb[:, t * hx:(t + 1) * hx],
                                     start=True, stop=True)
                    dst = hstr[:, j * hx:(j + 1) * hx]
                    if j % 2 == 0:
                        nc.scalar.copy(dst, hp[:])
                    else:
                        nc.vector.tensor_copy(dst, hp[:])
                out_ap = h_all.ap()[t * npadt + base:
                                    t * npadt + base + STRIPE * P, 0:hx]
                out_ap = out_ap.rearrange("(j p) k -> p j k", p=P)
                nc.sync.dma_start(out=out_ap, in_=hstr[:].rearrange(
                    "p (j k) -> p j k", k=hx))

        def superslot(ci):
            cblk = calls[ci]
            t = cblk["t"]
            base_row = t * npadt + cblk["src_half"] * nhblk * P
            lim = (nhblk * P if cblk["src_half"] == 0
                   else npadt - nhblk * P)
            nt_b = cblk["nt"]
            slot0 = cblk["slot0"]

            # ---- ed_blk for the slots: gather own-block rows, batched
            #      mult+reduce over the h columns
            it_b = gidx.tile([128, nt_b * P // 16], dt.int16, tag="itb")
            nc.sync.dma_start(
                out=it_b[:],
                in_=sidx.ap()[:, cblk["woff"]:cblk["woff"] + nt_b * P // 16])
            hb = gblk.tile([P, nt_b * ROWE], dt.bfloat16, tag="hb")
            nc.gpsimd.dma_gather(
                out_ap=hb[:].rearrange("p (j k) -> p j k", k=ROWE),
                in_ap=h_all.ap()[base_row:base_row + lim, :],
                idxs_ap=it_b[:], num_idxs=nt_b * P,
                num_idxs_reg=nt_b * P, elem_size=ROWE,
                single_packet=False)
            hb4 = hb[:].rearrange("p (s c) -> p s c", c=ROWE)
            tmpb = gsm.tile([P, nt_b * hk], dt.bfloat16, tag="tmpb")
            nc.vector.tensor_tensor(
                out=tmpb[:].rearrange("p (s h k) -> p s h k", h=heads, k=hd),
                in0=hb4[:, :, 0:hk].rearrange("p s (h k) -> p s h k", k=hd),
                in1=adb[:, t * hk:(t + 1) * hk]
                    .rearrange("p (h k) -> p h k", k=hd)
                    .unsqueeze(1).to_broadcast([P, nt_b, heads, hd]),
                op=OP.mult)
            edf = gsm.tile([P, nt_b * heads], dt.float32, tag="edf")
            nc.vector.tensor_reduce(
                out=edf[:].rearrange("p (s h) -> p s h", h=heads),
                in_=tmpb[:].rearrange("p (s h k) -> p s h k", h=heads, k=hd),
                axis=mybir.AxisListType.X, op=OP.add)
            edb = gsm.tile([P, nt_b * heads], dt.bfloat16, tag="edb")
            nc.vector.tensor_copy(edb[:], edf[:])

            # ---- edge calls of this superslot
            ss_edge = []
            cj = ci + 1
            while cj < len(calls) and calls[cj]["kind"] == "edge":
                ss_edge.append(calls[cj])
                cj += 1
            nt_ss = sum(cc["nt"] for cc in ss_edge)
            tile0_ss = ss_edge[0]["tile0"]

            hs = ghs.tile([P, nt_ss * ROWE], dt.bfloat16, tag="hs")
            off = 0
            for cc in ss_edge:
                base_e = t * npadt + cc["src_half"] * HALF
                lim_e = (min(HALF, npadt) if cc["src_half"] == 0
                         else npadt - HALF)
                nt = cc["nt"]
                it_e = gidx.tile([128, nt * P // 16], dt.int16, tag="ite",
                                 bufs=3)
                nc.sync.dma_start(
                    out=it_e[:],
                    in_=sidx.ap()[:, cc["woff"]:cc["woff"] + nt * P // 16])
                nc.gpsimd.dma_gather(
                    out_ap=hs[:, off * ROWE:(off + nt) * ROWE]
                        .rearrange("p (j k) -> p j k", k=ROWE),
                    in_ap=h_all.ap()[base_e:base_e + lim_e, :],
                    idxs_ap=it_e[:], num_idxs=nt * P,
                    num_idxs_reg=nt * P, elem_size=ROWE,
                    single_packet=False)
                off += nt
            hs4 = hs[:].rearrange("p (j c) -> p j c", c=ROWE)

            # ---- shipped one-hot sel for all tiles of the superslot
            sel = selp.tile([P, nt_ss * P], dt.bfloat16, tag="sel")
            nc.sync.dma_start(
                out=sel[:],
                in_=selin.ap()[:, tile0_ss * P:(tile0_ss + nt_ss) * P])

            # ---- per tile: transpose sel, ed matmul into z strip
            zps = psz.tile([P, nt_ss * heads], dt.float32, name="zps")
            for j in range(nt_ss):
                sid = tiles[tile0_ss + j][0]
                s_loc = sid - slot0
                stp = pst.tile([P, P], dt.bfloat16)
                nc.tensor.transpose(stp[:], sel[:, j * P:(j + 1) * P],
                                    idsb[:])
                sts = stsp.tile([P, P], dt.bfloat16, tag="sts")
                if j % 2 == 0:
                    nc.scalar.copy(sts[:], stp[:])
                else:
                    nc.vector.tensor_copy(sts[:], stp[:])
                nc.tensor.matmul(
                    zps[:, j * heads:(j + 1) * heads], sts[:],
                    edb[:, s_loc * heads:(s_loc + 1) * heads],
                    start=True, stop=True)

            # ---- batched alpha = exp(prelu(z + es, 0.2))
            zb = gsm.tile([P, nt_ss * heads], dt.float32, tag="zb")
            nc.vector.tensor_tensor(
                out=zb[:].rearrange("p (j h) -> p j h", h=heads),
                in0=zps[:].rearrange("p (j h) -> p j h", h=heads),
                in1=hs4[:, :, hk:hk + heads], op=OP.add)
            zl = gsm.tile([P, nt_ss * heads], dt.float32, tag="zl")
            nc.vector.scalar_tensor_tensor(
                out=zl[:], in0=zb[:], scalar=NEG_SLOPE, in1=zb[:],
                op0=OP.mult, op1=OP.max)
            ab = gsm.tile([P, nt_ss * heads], dt.bfloat16, tag="ab")
            nc.scalar.activation(ab[:], zl[:], AF.Exp)

            # ---- batched rhs build: [h*alpha | alpha]
            rhs = rhsp.tile([P, nt_ss * hx], dt.bfloat16, tag="rhs")
            rhs4 = rhs[:].rearrange("p (j c) -> p j c", c=hx)
            nc.vector.tensor_tensor(
                out=rhs4[:, :, 0:hk].rearrange("p j (h k) -> p j h k", k=hd),
                in0=hs4[:, :, 0:hk].rearrange("p j (h k) -> p j h k", k=hd),
                in1=ab[:].rearrange("p (j h) -> p j h", h=heads)
                    .unsqueeze(3).to_broadcast([P, nt_ss, heads, hd]),
                op=OP.mult)
            nc.vector.tensor_copy(
                rhs4[:, :, hk:hk + heads],
                ab[:].rearrange("p (j h) -> p j h", h=heads))

            # ---- per tile: aggregation matmul into per-slot psum region;
            #      grouped by slot so accumulation groups in a shared bank
            #      never overlap
            agg = psa.tile([P, SSG * 256], dt.float32, name="aggps")
            by_slot = [[] for _ in range(SSG)]
            for j in range(nt_ss):
                sid = tiles[tile0_ss + j][0]
                by_slot[sid - slot0].append(j)
            for s_loc in range(SSG):
                for j in by_slot[s_loc]:
                    _, first, last = tiles[tile0_ss + j]
                    nc.tensor.matmul(
                        agg[:, s_loc * 256:s_loc * 256 + hx],
                        sel[:, j * P:(j + 1) * P],
                        rhs4[:, j, :],
                        start=first, stop=last)

            # ---- batched finalize: out = elu(agg / (denom + 1e-9))
            agg4 = agg[:].rearrange("p (s c) -> p s c", c=256)
            dn = finp.tile([P, SSG * heads], dt.float32, tag="dn")
            nc.vector.tensor_scalar_add(
                dn[:].rearrange("p (s h) -> p s h", h=heads),
                agg4[:, :, hk:hk + heads], 1e-9)
            rc = finp.tile([P, SSG * heads], dt.float32, tag="rc")
            nc.vector.reciprocal(rc[:], dn[:])
            of = finp.tile([P, SSG * hk], dt.float32, tag="of")
            nc.vector.tensor_tensor(
                out=of[:].rearrange("p (s h k) -> p s h k", h=heads, k=hd),
                in0=agg4[:, :, 0:hk].rearrange("p s (h k) -> p s h k", k=hd),
                in1=rc[:].rearrange("p (s h) -> p s h", h=heads)
                    .unsqueeze(3).to_broadcast([P, SSG, heads, hd]),
                op=OP.mult)
            # elu(x) = (min(exp(x), 1) + max(x, 0)) - 1
            ex = finp.tile([P, SSG * hk], dt.float32, tag="ex")
            nc.scalar.activation(ex[:], of[:], AF.Exp)
            exm = finp.tile([P, SSG * hk], dt.float32, tag="exm")
            nc.vector.scalar_tensor_tensor(
                out=exm[:], in0=ex[:], scalar=1.0,
                in1=zerob[:].to_broadcast([P, SSG * hk]),
                op0=OP.min, op1=OP.add)
            s2 = finp.tile([P, SSG * hk], dt.float32, tag="s2")
            nc.vector.scalar_tensor_tensor(
                out=s2[:], in0=of[:], scalar=0.0, in1=exm[:],
                op0=OP.max, op1=OP.add)
            ysb = finp.tile([P, SSG * hk], dt.float32, tag="ysb")
            nc.vector.scalar_tensor_tensor(
                out=ysb[:], in0=s2[:], scalar=-1.0,
                in1=zerob[:].to_broadcast([P, SSG * hk]),
                op0=OP.add, op1=OP.add)
            nc.sync.dma_start(
                out=ycat.ap()[slot0 * P:(slot0 + SSG) * P, :]
                    .rearrange("(s p) k -> p s k", p=P),
                in_=ysb[:].rearrange("p (s k) -> p s k", k=hk))
            return cj

        # interleave: phase 1 of type t, then its superslots (types are
        # contiguous in the call stream)
        ci = 0
        for t in range(ntypes):
            phase1(t)
            while ci < len(calls) and calls[ci]["t"] == t:
                ci = superslot(ci)
        assert ci == len(calls)

    nc.compile()
    return nc


# ----------------------------------------------------------------------------
# public entry
# ----------------------------------------------------------------------------

def _run(embedding, edges, W, a_src, a_dst, ncores=8, sim=False, trace=False):
    embedding = np.asarray(embedding, np.float32)
    edges = np.asarray(edges, np.int32)
    W = np.asarray(W, np.float32)
    a_src = np.asarray(a_src, np.float32)
    a_dst = np.asarray(a_dst, np.float32)

    n, d = embedding.shape
    ntypes = edges.shape[0]
    heads, hd = a_src.shape[1], a_src.shape[2]

    plan = _plan(edges, n, ncores)
    xT, Wmx, adr, ident = _host_tensors(embedding, W, a_src, a_dst, plan)
    nc = _build_program(plan, d, heads, hd)

    in_maps = []
    for c in range(ncores):
        in_maps.append({
            "xT": xT, "Wmx": Wmx, "adr": adr, "ident": ident,
            "sidx": plan["sidx16"][c], "selin": plan["selhost"][c],
        })

    if sim:
        from concourse.bass_interp import CoreSim
        results = []
        for c in range(ncores):
            s = CoreSim(nc)
            for k, v in in_maps[c].items():
                s.tensor(k)[:] = v
            s.simulate()
            results.append({"ycat": np.array(s.tensor("ycat"))})
        exec_ns = None
    else:
        from concourse.bass_utils import run_bass_kernel_spmd
        r = run_bass_kernel_spmd(nc, in_maps, core_ids=list(range(ncores)),
                                 trace=trace)
        results = r.results
        exec_ns = r.exec_time_ns
        if trace:
            _TRACE[0] = r

    out = np.zeros((ntypes, n, heads * hd), np.float32)
    for c in range(ncores):
        y = results[c]["ycat"]
        for sid, tb in enumerate(plan["outmap"][c]):
            if tb is None:
                continue
            t, b = tb
            lo = b * P
            hi = min(n, lo + P)
            out[t, lo:hi, :] = y[sid * P:sid * P + (hi - lo), :]
    return out, exec_ns


_EXEC_NS = [None]
_TRACE = [None]


def kernel(embedding, edges, W, a_src, a_dst):
    out, exec_ns = _run(embedding, edges, W, a_src, a_dst, ncores=8, sim=False)
    _EXEC_NS[0] = exec_ns
    return out, out.copy()


# revision 15
# speedup vs baseline: 1.2793x; 1.0089x over previous
"""Multi-type GAT (node-level attention) kernel for Trainium2, 8 NeuronCores.

Strategy (graph partitioned by destination-node blocks of 128):
  * Host: per edge type, bucket edges by dst block (stable sort); within each
    bucket split edges by src half (< 32768) so every dma_gather call uses
    int16 indices into one half-table; assign buckets to cores balanced by
    tile count (LPT) within each (type, dst-half) group; build a uniform
    compile-time schedule so all 8 cores run one program.  The per-tile
    one-hot sel matrices (dst-local routing) are also host-built and shipped
    as bf16 inputs.
  * Device phase 1 (per type, interleaved with that type's phase 2):
    [h | es] = x @ [W | W a_src] per node tile on PE, rows stored bf16 to an
    internal DRAM table h_all[3*npadt, 256] (512B pitch).
  * Device phase 2, per superslot (4 dst-block slots, ~60-70 edge tiles):
      - dma_gather the 4 blocks' own rows; ONE batched mult+reduce gives
        ed_blk for all 4 slots
      - dma_gather [h|es][src] rows (512B each, by src half) for all tiles
      - per tile: PE-transpose the shipped sel, tiny matmul sts^T @ ed_blk
        accumulates ed per edge into one PSUM z strip [128, nt*4]
      - batched z+es add, Scalar-engine Prelu(0.2) and Exp
      - ONE batched rhs build (h*alpha | alpha) for all tiles
      - per tile: matmul psum[slot] += sel^T @ rhs, accumulated per slot
      - batched finalize: out = elu(agg / (denom + 1e-9)) for all 4 slots,
        single contiguous write per superslot
  * Host: unpermute slot-order rows back to node order.

The reference module computes the identical GAT stack twice (gat + gcn
branches), so the kernel computes once and returns the array twice.
"""

from contextlib import ExitStack

import numpy as np
import ml_dtypes

BF16 = ml_dtypes.bfloat16
FP8 = ml_dtypes.float8_e4m3

P = 128
NEG_SLOPE = 0.2
HALF = 32768     # int16-addressable rows per gather table
SSG = 4          # buckets (slots) per superslot
STRIPE = 8       # node tiles per phase-1 stripe
ROWE = 256       # gather-row elements (bf16): [h 128 | es 4 | pad]


def _wrap_idx(vals):
    """dma_gather index packing: index i -> partition i%16, col i//16,
    replicated across the 8 groups of 16 partitions."""
    vals = np.asarray(vals, np.int16)
    assert len(vals) % 16 == 0
    w = vals.reshape(-1, 16).T
    return np.tile(w, (8, 1))


# ----------------------------------------------------------------------------
# host-side planning
# ----------------------------------------------------------------------------

def _plan(edges: np.ndarray, n_nodes: int, ncores: int):
    ntypes = edges.shape[0]
    nblk = (n_nodes + P - 1) // P
    npadt = ((nblk + STRIPE - 1) // STRIPE) * STRIPE * P
    nhblk = min(HALF // P, nblk)          # dst blocks in half 0

    # group buckets by (type, dst half); per bucket: src list split by src half
    groups = {}
    for t in range(ntypes):
        src = np.asarray(edges[t, 0], np.int64)
        dst = np.asarray(edges[t, 1], np.int64)
        blk = dst // P
        order = np.argsort(blk, kind="stable")
        bs, ss, ds_ = blk[order], src[order], dst[order]
        dl = ds_ - bs * P
        starts = np.searchsorted(bs, np.arange(nblk), "left")
        ends = np.searchsorted(bs, np.arange(nblk), "right")
        for bh in range(2):
            groups[(t, bh)] = []
        for b in range(nblk):
            sl = slice(starts[b], ends[b])
            sb, db = ss[sl], dl[sl]
            ah = sb < HALF
            bh = 0 if b < nhblk else 1
            groups[(t, bh)].append(
                (b, sb[ah], db[ah], sb[~ah] - HALF, db[~ah]))

    # LPT per group, then uniform schedule of (tA, tB) per rank
    plan_groups = []
    slot_id = 0
    outmap = [[] for _ in range(ncores)]
    for (t, bh), buckets in sorted(groups.items()):
        wt = [((len(x[1]) + P - 1) // P + (len(x[3]) + P - 1) // P)
              for x in buckets]
        order = np.argsort(-np.asarray(wt), kind="stable")
        cs = [[] for _ in range(ncores)]
        load = np.zeros(ncores, np.int64)
        for i in order:
            c = int(np.argmin(load))
            cs[c].append(int(i))
            load[c] += max(1, wt[i])
        S = max(len(x) for x in cs)
        S = ((S + SSG - 1) // SSG) * SSG
        ranks = []
        for r in range(S):
            ta = tb = 0
            for c in range(ncores):
                if r < len(cs[c]):
                    x = buckets[cs[c][r]]
                    ta = max(ta, (len(x[1]) + P - 1) // P)
                    tb = max(tb, (len(x[3]) + P - 1) // P)
            if ta + tb == 0:
                ta = 1
            ranks.append((ta, tb))
        for c in range(ncores):
            for r in range(S):
                if r < len(cs[c]):
                    outmap[c].append((t, buckets[cs[c][r]][0]))
                else:
                    outmap[c].append(None)
        plan_groups.append(dict(t=t, bh=bh, S=S, ranks=ranks, cs=cs,
                                buckets=buckets, slot0=slot_id))
        slot_id += S
    S_total = slot_id

    # compile-time tile stream + calls; per-core data arrays
    tiles = []      # (slot_id, first, last)
    calls = []      # dict(kind, t, src_half, num_idxs, woff, tile0)
    woff = 0        # int16 index-array column offset
    tile0 = 0
    core_idx = [[] for _ in range(ncores)]   # int16 stream per core
    core_blk = [[] for _ in range(ncores)]   # block-row idx stream
    core_dloc = [np.full((0,), 300.0, np.float32) for _ in range(ncores)]

    for g in plan_groups:
        t, bh, S, ranks, cs, buckets = (g["t"], g["bh"], g["S"], g["ranks"],
                                        g["cs"], g["buckets"])
        base_blk = 0 if bh == 0 else nhblk * P
        for s0 in range(0, S, SSG):
            rr = list(range(s0, min(s0 + SSG, S)))
            # block-row gather call for ed_blk (relative to dst-half base)
            calls.append(dict(kind="blk", t=t, src_half=bh,
                              num_idxs=len(rr) * P, woff=woff,
                              tile0=tile0, nt=len(rr),
                              slot0=g["slot0"] + s0))
            woff += len(rr) * P // 16
            for c in range(ncores):
                for r in rr:
                    if r < len(cs[c]):
                        b = buckets[cs[c][r]][0]
                        rel = b * P - base_blk
                    else:
                        rel = 0
                    core_blk[c].extend(range(rel, rel + P))
            for half, wcol in ((0, 1), (1, 3)):
                nt = sum(ranks[r][half] for r in rr)
                if nt == 0:
                    continue
                calls.append(dict(kind="edge", t=t, src_half=half,
                                  num_idxs=nt * P, woff=woff, tile0=tile0,
                                  nt=nt))
                woff += nt * P // 16
                for c in range(ncores):
                    seg_i = np.zeros(nt * P, np.int64)
                    seg_d = np.full(nt * P, 300.0, np.float32)
                    pos = 0
                    for r in rr:
                        trk = ranks[r][half]
                        if r < len(cs[c]):
                            x = buckets[cs[c][r]]
                            sv, dv = x[wcol], x[wcol + 1]
                            seg_i[pos:pos + len(sv)] = sv
                            seg_d[pos:pos + len(sv)] = dv
                        pos += trk * P
                    core_idx[c].append(seg_i)
                    core_dloc[c] = np.concatenate([core_dloc[c], seg_d])
                # tile bookkeeping
                for r in rr:
                    for j in range(ranks[r][half]):
                        sid = g["slot0"] + r
                        first = (half == 0 or ranks[r][0] == 0) and j == 0
                        last = ((half == 1 or ranks[r][1] == 0)
                                and j == ranks[r][half] - 1)
                        tiles.append((sid, first, last))
                        tile0 += 1

    tot_tiles = tile0
    W_total = woff

    # pack per-core arrays: gather indices + host-built one-hot sel
    sidx16 = np.zeros((ncores, 128, W_total), np.int16)
    selhost = np.zeros((ncores, 128, tot_tiles * P), FP8)
    selhostT = np.zeros((ncores, 128, tot_tiles * P), FP8)
    for c in range(ncores):
        ei = 0
        blk_arr = np.asarray(core_blk[c], np.int64)
        bpos = 0
        for call in calls:
            n = call["num_idxs"]
            if call["kind"] == "blk":
                vals = blk_arr[bpos:bpos + n]
                bpos += n
            else:
                vals = core_idx[c][ei]
                ei += 1
            sidx16[c, :, call["woff"]:call["woff"] + n // 16] = _wrap_idx(vals)
        d = core_dloc[c].reshape(tot_tiles, P).astype(np.int64)
        oh = np.zeros((tot_tiles, P, P), FP8)
        ti, pp = np.nonzero((d >= 0) & (d < P))
        oh[ti, pp, d[ti, pp]] = 1.0
        selhost[c] = oh.transpose(1, 0, 2).reshape(P, tot_tiles * P)
        selhostT[c] = oh.transpose(2, 0, 1).reshape(P, tot_tiles * P)

    # max tiles in any superslot (PSUM z strip must fit one bank)
    nt_ss_max = 0
    i0 = 0
    while i0 < len(calls):
        assert calls[i0]["kind"] == "blk"
        j0 = i0 + 1
        acc = 0
        while j0 < len(calls) and calls[j0]["kind"] == "edge":
            acc += calls[j0]["nt"]
            j0 += 1
        nt_ss_max = max(nt_ss_max, acc)
        i0 = j0
    assert nt_ss_max * 4 <= 512, "z strip must fit one PSUM bank"

    return dict(ntypes=ntypes, nblk=nblk, npadt=npadt, nhblk=nhblk,
                S_total=S_total, tot_tiles=tot_tiles, W_total=W_total,
                tiles=tiles, calls=calls, outmap=outmap,
                sidx16=sidx16, selhost=selhost, selhostT=selhostT,
                nt_ss_max=nt_ss_max)


def _host_tensors(embedding, W, a_src, a_dst, plan):
    n, d = embedding.shape
    ntypes = W.shape[0]
    heads, hd = a_src.shape[1], a_src.shape[2]
    hk = heads * hd
    npadt = plan["npadt"]

    xT = np.zeros((d, npadt), np.float32)
    xT[:, :n] = np.asarray(embedding, np.float32).T
    xT = xT.astype(BF16)

    # Wmx[:, t*(hk+heads) : ...] = [W_t | W_t @ a_src_blockdiag]
    Wf = np.asarray(W, np.float32).reshape(ntypes, d, heads, hd)
    Was = np.einsum("tdhk,thk->tdh", Wf, np.asarray(a_src, np.float32))
    Wmx = np.concatenate(
        [Wf.reshape(ntypes, d, hk), Was], axis=2)      # [t, d, hk+heads]
    Wmx = np.ascontiguousarray(
        Wmx.transpose(1, 0, 2).reshape(d, ntypes * (hk + heads))).astype(BF16)

    adr = np.broadcast_to(
        np.asarray(a_dst, np.float32).reshape(ntypes, hk)
        .reshape(1, ntypes * hk), (P, ntypes * hk))
    adr = np.ascontiguousarray(adr).astype(BF16)

    ident = np.eye(P, dtype=np.float32).astype(BF16)
    return xT, Wmx, adr, ident


def _gather_compact(nc, mybir, out_ap, in_ap, idxs_ap, num_idxs, elem_size,
                    elem_step):
    """dma_gather with elem_size not a multiple of 256B (non-transpose,
    DRAM source). Mirrors BassGpSimd.dma_gather minus the transpose-only
    elem-size assert; row pitch (elem_step bytes) must stay 256B-aligned."""
    gp = nc.gpsimd
    assert idxs_ap.dtype == mybir.dt.int16
    dts = mybir.dt.size(in_ap.dtype)
    assert in_ap.ap[-1][1] == out_ap.ap[-1][1] == elem_size
    assert out_ap.ap[0][1] * out_ap.ap[1][1] == ((num_idxs + 127) // 128) * 128
    assert in_ap.ap[0][0] == elem_step
    stride_bytes = elem_step * dts
    assert stride_bytes % 256 == 0
    _in_ap = gp.lower_ap_dma(in_ap, for_custom_bir_dma=True)
    _idxs_ap = gp.lower_ap(idxs_ap)
    _out_ap = gp.lower_ap(out_ap)
    return gp.add_instruction(
        mybir.InstDMAGatherAnt(
            name=gp.bass.get_next_instruction_name(),
            ins=[*_in_ap, _idxs_ap,
                 gp.lower_val_access(gp.to_reg(num_idxs))],
            outs=[_out_ap],
            transpose=False,
            num_idxs=num_idxs,
            elem_size=elem_size,
            stride_bytes_256=stride_bytes // 256,
            gen_mode=0,
            single_packet=False,
            queue_num=0,
            sbuf_tokens_per_rank=0,
            sbuf_free_dim_per_rank=0,
            sbuf_free_dim_pad_per_rank=0,
            sbuf_byte_offset=0,
        )
    )


# ----------------------------------------------------------------------------
# device program
# ----------------------------------------------------------------------------

def _build_program(plan, d, heads, hd):
    import concourse.bacc as bacc
    import concourse.tile as tile
    import concourse.mybir as mybir

    dt = mybir.dt
    ntypes = plan["ntypes"]
    npadt = plan["npadt"]
    hk = heads * hd  # 128
    hx = hk + heads  # 132

    nc = bacc.Bacc("TRN2", target_bir_lowering=False, debug=False,
                   enable_asserts=False, num_devices=1)

    xT = nc.dram_tensor("xT", (d, npadt), dt.bfloat16, kind="ExternalInput")
    Wmx = nc.dram_tensor("Wmx", (d, ntypes * hx), dt.bfloat16,
                         kind="ExternalInput")
    adr = nc.dram_tensor("adr", (P, ntypes * hk), dt.bfloat16,
                         kind="ExternalInput")
    ident = nc.dram_tensor("ident", (P, P), dt.bfloat16, kind="ExternalInput")
    sidx = nc.dram_tensor("sidx", (128, plan["W_total"]), dt.int16,
                          kind="ExternalInput")
    selin = nc.dram_tensor("selin", (128, plan["tot_tiles"] * P),
                           dt.float8e4, kind="ExternalInput")
    selinT = nc.dram_tensor("selinT", (128, plan["tot_tiles"] * P),
                            dt.float8e4, kind="ExternalInput")
    h_all = nc.dram_tensor("h_all", (ntypes * npadt, ROWE), dt.bfloat16,
                           kind="Internal")
    ycat = nc.dram_tensor("ycat", (plan["S_total"] * P, hk), dt.float32,
                          kind="ExternalOutput")

    nstripes = npadt // (STRIPE * P)
    AF = mybir.ActivationFunctionType
    OP = mybir.AluOpType
    tiles = plan["tiles"]
    nhblk = plan["nhblk"]
    calls = plan["calls"]

    with tile.TileContext(nc) as tc, ExitStack() as ctx:
        consts = ctx.enter_context(tc.tile_pool(name="consts", bufs=1))
        wsb = consts.tile([d, ntypes * hx], dt.bfloat16)
        nc.sync.dma_start(out=wsb[:], in_=Wmx.ap())
        adb = consts.tile([P, ntypes * hk], dt.bfloat16)
        nc.sync.dma_start(out=adb[:], in_=adr.ap())
        idsb = consts.tile([P, P], dt.bfloat16)
        nc.sync.dma_start(out=idsb[:], in_=ident.ap())
        zerob = consts.tile([P, 1], dt.float32)
        nc.vector.memset(zerob[:], 0.0)

        p1x = ctx.enter_context(tc.tile_pool(name="p1x", bufs=2))
        p1h = ctx.enter_context(tc.tile_pool(name="p1h", bufs=2))
        p1ps = ctx.enter_context(tc.tile_pool(name="p1ps", bufs=2,
                                              space="PSUM"))
        gidx = ctx.enter_context(tc.tile_pool(name="gidx", bufs=3))
        ghs = ctx.enter_context(tc.tile_pool(name="ghs", bufs=2))
        gblk = ctx.enter_context(tc.tile_pool(name="gblk", bufs=2))
        gsm = ctx.enter_context(tc.tile_pool(name="gsm", bufs=2))
        selp = ctx.enter_context(tc.tile_pool(name="selp", bufs=2))
        rhsp = ctx.enter_context(tc.tile_pool(name="rhsp", bufs=2))
        finp = ctx.enter_context(tc.tile_pool(name="finp", bufs=1))
        psz = ctx.enter_context(tc.tile_pool(name="psz", bufs=2, space="PSUM"))
        psa = ctx.enter_context(tc.tile_pool(name="psa", bufs=1, space="PSUM"))

        def phase1(t):
            for st in range(nstripes):
                base = st * STRIPE * P
                xt = p1x.tile([d, STRIPE * P], dt.bfloat16, tag="xt")
                nc.sync.dma_start(out=xt[:],
                                  in_=xT.ap()[:, base:base + STRIPE * P])
                hstr = p1h.tile([P, STRIPE * hx], dt.bfloat16, tag="hstr")
                for j in range(STRIPE):
                    lhs = xt[:, j * P:(j + 1) * P]
                    hp = p1ps.tile([P, hx], dt.float32)
                    nc.tensor.matmul(hp[:], lhs,
                                     wsb[:, t * hx:(t + 1) * hx],
                                     start=True, stop=True)
                    dst = hstr[:, j * hx:(j + 1) * hx]
                    if j % 2 == 0:
                        nc.scalar.copy(dst, hp[:])
                    else:
                        nc.vector.tensor_copy(dst, hp[:])
                out_ap = h_all.ap()[t * npadt + base:
                                    t * npadt + base + STRIPE * P, 0:hx]
                out_ap = out_ap.rearrange("(j p) k -> p j k", p=P)
                nc.sync.dma_start(out=out_ap, in_=hstr[:].rearrange(
                    "p (j k) -> p j k", k=hx))

        def superslot(ci):
            cblk = calls[ci]
            t = cblk["t"]
            base_row = t * npadt + cblk["src_half"] * nhblk * P
            lim = (nhblk * P if cblk["src_half"] == 0
                   else npadt - nhblk * P)
            nt_b = cblk["nt"]
            slot0 = cblk["slot0"]

            # ---- ed_blk for the slots: gather own-block rows, batched
            #      mult+reduce over the h columns
            it_b = gidx.tile([128, nt_b * P // 16], dt.int16, tag="itb")
            nc.sync.dma_start(
                out=it_b[:],
                in_=sidx.ap()[:, cblk["woff"]:cblk["woff"] + nt_b * P // 16])
            hb = gblk.tile([P, nt_b * ROWE], dt.bfloat16, tag="hb")
            nc.gpsimd.dma_gather(
                out_ap=hb[:].rearrange("p (j k) -> p j k", k=ROWE),
                in_ap=h_all.ap()[base_row:base_row + lim, :],
                idxs_ap=it_b[:], num_idxs=nt_b * P,
                num_idxs_reg=nt_b * P, elem_size=ROWE,
                single_packet=False)
            hb4 = hb[:].rearrange("p (s c) -> p s c", c=ROWE)
            tmpb = gsm.tile([P, nt_b * hk], dt.bfloat16, tag="tmpb")
            nc.vector.tensor_tensor(
                out=tmpb[:].rearrange("p (s h k) -> p s h k", h=heads, k=hd),
                in0=hb4[:, :, 0:hk].rearrange("p s (h k) -> p s h k", k=hd),
                in1=adb[:, t * hk:(t + 1) * hk]
                    .rearrange("p (h k) -> p h k", k=hd)
                    .unsqueeze(1).to_broadcast([P, nt_b, heads, hd]),
                op=OP.mult)
            edf = gsm.tile([P, nt_b * heads], dt.float32, tag="edf")
            nc.vector.tensor_reduce(
                out=edf[:].rearrange("p (s h) -> p s h", h=heads),
                in_=tmpb[:].rearrange("p (s h k) -> p s h k", h=heads, k=hd),
                axis=mybir.AxisListType.X, op=OP.add)
            edb = gsm.tile([P, nt_b * heads], dt.bfloat16, tag="edb")
            nc.vector.tensor_copy(edb[:], edf[:])

            # ---- edge calls of this superslot
            ss_edge = []
            cj = ci + 1
            while cj < len(calls) and calls[cj]["kind"] == "edge":
                ss_edge.append(calls[cj])
                cj += 1
            nt_ss = sum(cc["nt"] for cc in ss_edge)
            tile0_ss = ss_edge[0]["tile0"]

            hs = ghs.tile([P, nt_ss * ROWE], dt.bfloat16, tag="hs")
            off = 0
            for cc in ss_edge:
                base_e = t * npadt + cc["src_half"] * HALF
                lim_e = (min(HALF, npadt) if cc["src_half"] == 0
                         else npadt - HALF)
                nt = cc["nt"]
                it_e = gidx.tile([128, nt * P // 16], dt.int16, tag="ite",
                                 bufs=3)
                nc.sync.dma_start(
                    out=it_e[:],
                    in_=sidx.ap()[:, cc["woff"]:cc["woff"] + nt * P // 16])
                nc.gpsimd.dma_gather(
                    out_ap=hs[:, off * ROWE:(off + nt) * ROWE]
                        .rearrange("p (j k) -> p j k", k=ROWE),
                    in_ap=h_all.ap()[base_e:base_e + lim_e, :],
                    idxs_ap=it_e[:], num_idxs=nt * P,
                    num_idxs_reg=nt * P, elem_size=ROWE,
                    single_packet=False)
                off += nt
            hs4 = hs[:].rearrange("p (j c) -> p j c", c=ROWE)

            # ---- shipped one-hot sel / selT for all tiles
            sel = selp.tile([P, nt_ss * P], dt.float8e4, tag="sel")
            nc.sync.dma_start(
                out=sel[:],
                in_=selin.ap()[:, tile0_ss * P:(tile0_ss + nt_ss) * P])
            selT = selp.tile([P, nt_ss * P], dt.float8e4, tag="selT")
            nc.sync.dma_start(
                out=selT[:],
                in_=selinT.ap()[:, tile0_ss * P:(tile0_ss + nt_ss) * P])

            # ---- per tile: ed matmul (lhsT = shipped selT) into z strip
            zps = psz.tile([P, nt_ss * heads], dt.float32, name="zps")
            for j in range(nt_ss):
                sid = tiles[tile0_ss + j][0]
                s_loc = sid - slot0
                nc.tensor.matmul(
                    zps[:, j * heads:(j + 1) * heads],
                    selT[:, j * P:(j + 1) * P],
                    edb[:, s_loc * heads:(s_loc + 1) * heads],
                    start=True, stop=True)

            # ---- batched alpha = exp(prelu(z + es, 0.2))
            zb = gsm.tile([P, nt_ss * heads], dt.float32, tag="zb")
            nc.vector.tensor_tensor(
                out=zb[:].rearrange("p (j h) -> p j h", h=heads),
                in0=zps[:].rearrange("p (j h) -> p j h", h=heads),
                in1=hs4[:, :, hk:hk + heads], op=OP.add)
            zl = gsm.tile([P, nt_ss * heads], dt.float32, tag="zl")
            nc.vector.scalar_tensor_tensor(
                out=zl[:], in0=zb[:], scalar=NEG_SLOPE, in1=zb[:],
                op0=OP.mult, op1=OP.max)
            ab = gsm.tile([P, nt_ss * heads], dt.bfloat16, tag="ab")
            nc.scalar.activation(ab[:], zl[:], AF.Exp)

            # ---- batched rhs build: [h*alpha | alpha]
            rhs = rhsp.tile([P, nt_ss * hx], dt.bfloat16, tag="rhs")
            rhs4 = rhs[:].rearrange("p (j c) -> p j c", c=hx)
            nc.vector.tensor_tensor(
                out=rhs4[:, :, 0:hk].rearrange("p j (h k) -> p j h k", k=hd),
                in0=hs4[:, :, 0:hk].rearrange("p j (h k) -> p j h k", k=hd),
                in1=ab[:].rearrange("p (j h) -> p j h", h=heads)
                    .unsqueeze(3).to_broadcast([P, nt_ss, heads, hd]),
                op=OP.mult)
            nc.vector.tensor_copy(
                rhs4[:, :, hk:hk + heads],
                ab[:].rearrange("p (j h) -> p j h", h=heads))

            # ---- per tile: aggregation matmul into per-slot psum region;
            #      grouped by slot so accumulation groups in a shared bank
            #      never overlap
            agg = psa.tile([P, SSG * 256], dt.float32, name="aggps")
            by_slot = [[] for _ in range(SSG)]
            for j in range(nt_ss):
                sid = tiles[tile0_ss + j][0]
                by_slot[sid - slot0].append(j)
            for s_loc in range(SSG):
                for j in by_slot[s_loc]:
                    _, first, last = tiles[tile0_ss + j]
                    nc.tensor.matmul(
                        agg[:, s_loc * 256:s_loc * 256 + hx],
                        sel[:, j * P:(j + 1) * P],
                        rhs4[:, j, :],
                        start=first, stop=last)

            # ---- batched finalize: out = elu(agg / (denom + 1e-9))
            agg4 = agg[:].rearrange("p (s c) -> p s c", c=256)
            dn = finp.tile([P, SSG * heads], dt.float32, tag="dn")
            nc.vector.tensor_scalar_add(
                dn[:].rearrange("p (s h) -> p s h", h=heads),
                agg4[:, :, hk:hk + heads], 1e-9)
            rc = finp.tile([P, SSG * heads], dt.float32, tag="rc")
            nc.vector.reciprocal(rc[:], dn[:])
            of = finp.tile([P, SSG * hk], dt.float32, tag="of")
            nc.vector.tensor_tensor(
                out=of[:].rearrange("p (s h k) -> p s h k", h=heads, k=hd),
                in0=agg4[:, :, 0:hk].rearrange("p s (h k) -> p s h k", k=hd),
                in1=rc[:].rearrange("p (s h) -> p s h", h=heads)
                    .unsqueeze(3).to_broadcast([P, SSG, heads, hd]),
                op=OP.mult)
            # elu(x) = (min(exp(x), 1) + max(x, 0)) - 1
            ex = finp.tile([P, SSG * hk], dt.float32, tag="ex")
            nc.scalar.activation(ex[:], of[:], AF.Exp)
            exm = finp.tile([P, SSG * hk], dt.float32, tag="exm")
            nc.vector.scalar_tensor_tensor(
                out=exm[:], in0=ex[:], scalar=1.0,
                in1=zerob[:].to_broadcast([P, SSG * hk]),
                op0=OP.min, op1=OP.add)
            s2 = finp.tile([P, SSG * hk], dt.float32, tag="s2")
            nc.vector.scalar_tensor_tensor(
                out=s2[:], in0=of[:], scalar=0.0, in1=exm[:],
                op0=OP.max, op1=OP.add)
            ysb = finp.tile([P, SSG * hk], dt.float32, tag="ysb")
            nc.vector.scalar_tensor_tensor(
                out=ysb[:], in0=s2[:], scalar=-1.0,
                in1=zerob[:].to_broadcast([P, SSG * hk]),
                op0=OP.add, op1=OP.add)
            nc.sync.dma_start(
                out=ycat.ap()[slot0 * P:(slot0 + SSG) * P, :]
                    .rearrange("(s p) k -> p s k", p=P),
                in_=ysb[:].rearrange("p (s k) -> p s k", k=hk))
            return cj

        # interleave: phase 1 of type t, then its superslots (types are
        # contiguous in the call stream)
        ci = 0
        for t in range(ntypes):
            phase1(t)
            while ci < len(calls) and calls[ci]["t"] == t:
                ci = superslot(ci)
        assert ci == len(calls)

    nc.compile()
    return nc


# ----------------------------------------------------------------------------
# public entry
# ----------------------------------------------------------------------------

def _run(embedding, edges, W, a_src, a_dst, ncores=8, sim=False, trace=False):
    embedding = np.asarray(embedding, np.float32)
    edges = np.asarray(edges, np.int32)
    W = np.asarray(W, np.float32)
    a_src = np.asarray(a_src, np.float32)
    a_dst = np.asarray(a_dst, np.float32)

    n, d = embedding.shape
    ntypes = edges.shape[0]
    heads, hd = a_src.shape[1], a_src.shape[2]

    plan = _plan(edges, n, ncores)
    xT, Wmx, adr, ident = _host_tensors(embedding, W, a_src, a_dst, plan)
    nc = _build_program(plan, d, heads, hd)

    in_maps = []
    for c in range(ncores):
        in_maps.append({
            "xT": xT, "Wmx": Wmx, "adr": adr, "ident": ident,
            "sidx": plan["sidx16"][c], "selin": plan["selhost"][c],
            "selinT": plan["selhostT"][c],
        })

    if sim:
        from concourse.bass_interp import CoreSim
        results = []
        for c in range(ncores):
            s = CoreSim(nc)
            for k, v in in_maps[c].items():
                s.tensor(k)[:] = v
            s.simulate()
            results.append({"ycat": np.array(s.tensor("ycat"))})
        exec_ns = None
    else:
        from concourse.bass_utils import run_bass_kernel_spmd
        r = run_bass_kernel_spmd(nc, in_maps, core_ids=list(range(ncores)),
                                 trace=trace)
        results = r.results
        exec_ns = r.exec_time_ns
        if trace:
            _TRACE[0] = r

    out = np.zeros((ntypes, n, heads * hd), np.float32)
    for c in range(ncores):
        y = results[c]["ycat"]
        for sid, tb in enumerate(plan["outmap"][c]):
            if tb is None:
                continue
            t, b = tb
            lo = b * P
            hi = min(n, lo + P)
            out[t, lo:hi, :] = y[sid * P:sid * P + (hi - lo), :]
    return out, exec_ns


_EXEC_NS = [None]
_TRACE = [None]


def kernel(embedding, edges, W, a_src, a_dst):
    out, exec_ns = _run(embedding, edges, W, a_src, a_dst, ncores=8, sim=False)
    _EXEC_NS[0] = exec_ns
    return out, out.copy()
